# revision 1
# baseline (speedup 1.0000x reference)
"""MDRNN 2D-grid recurrence kernel for 8 Trainium2 NeuronCores.

h[i,j] = tanh(x[i,j] @ w + h[i-1,j]*u0 + h[i,j-1]*u1 + bias)

Strategy:
  - Data-parallel over batch: B=16 -> 2 batch elements per core.
  - Host pre-transposes x into anti-diagonal-ordered [SIN+1, cells*b]
    layout (ones row appended so the GEMM adds the bias in PSUM).
  - GEMM (w stationary) runs ahead of the wavefront in PSUM chunks that
    are aligned to whole diagonals (<=512 cols); the wavefront reads a'
    straight out of PSUM.
  - Per anti-diagonal d: two fused scalar_tensor_tensor ops on DVE
    (t1 = h_left*u1 + a'; z = h_up*u0 + t1) and one ACT tanh that
    writes a packed staging buffer with one zero "gap" pair between
    diagonals -- the gaps provide the recurrence boundary zeros, so
    consecutive diagonals read each other with plain contiguous slices.
  - Output DMA is batched: one DMA per 2048-col staging segment.
  - Host inverse-permutes the gap-padded diag-ordered output to
    (i,j,b,o).
"""

import numpy as np

D1, D2, B, SIN, SOUT = 128, 128, 16, 64, 128
NCORES = 8
BLOC = B // NCORES  # 2
NCELLS = D1 * D2
NCOLS = NCELLS * BLOC  # 32768
ND = D1 + D2 - 1  # 255
GAP = BLOC  # one zero cell-pair between diagonals
NCOLS_G = NCOLS + GAP * ND + GAP  # 33280: staging/out cols incl gaps
SEG = 2048  # output DMA segment (cols)
NSEG = (NCOLS_G + SEG - 1) // SEG  # 17 (last partial)


def _diag_geom():
    """Per-diag (i0, C); packed bases; gapped bases."""
    geo, bases, gbases = [], [0], [GAP]
    for d in range(ND):
        i0 = max(0, d - (D2 - 1))
        i1 = min(D1 - 1, d)
        C = i1 - i0 + 1
        geo.append((i0, C))
        bases.append(bases[-1] + C)
        gbases.append(gbases[-1] + C * BLOC + GAP)
    return geo, bases, gbases


def _diag_order():
    I, J = [], []
    for d in range(ND):
        for i in range(max(0, d - (D2 - 1)), min(D1 - 1, d) + 1):
            I.append(i)
            J.append(d - i)
    return np.array(I), np.array(J)


_CACHE = {}


def _build_program():
    if "nc" in _CACHE:
        return _CACHE["nc"]
    import concourse.mybir as mybir
    from concourse import bacc
    import concourse.bass as bass
    from concourse.tile import TileContext

    f32 = mybir.dt.float32
    mult = mybir.AluOpType.mult
    add = mybir.AluOpType.add
    Tanh = mybir.ActivationFunctionType.Tanh

    geo, bases, gbases = _diag_geom()

    # GEMM chunks = greedy groups of whole diagonals, <=512 cols each.
    chunks = []  # (start_diag, end_diag, col0, ncols)
    d0 = 0
    while d0 < ND:
        col0 = bases[d0] * BLOC
        d1 = d0
        while d1 + 1 < ND and (bases[d1 + 2] * BLOC - col0) <= 512:
            d1 += 1
        chunks.append((d0, d1, col0, bases[d1 + 1] * BLOC - col0))
        d0 = d1 + 1
    chunk_of_diag = {}
    for ci, (a, b, _, _) in enumerate(chunks):
        for d in range(a, b + 1):
            chunk_of_diag[d] = ci

    nc = bacc.Bacc(None, target_bir_lowering=False)
    xa = nc.dram_tensor("xa", (SIN + 1, NCOLS), f32, kind="ExternalInput")
    wb = nc.dram_tensor("wb", (SIN + 1, SOUT), f32, kind="ExternalInput")
    uu = nc.dram_tensor("uu", (SOUT, 2), f32, kind="ExternalInput")
    ho = nc.dram_tensor("ho", (SOUT, NCOLS_G), f32, kind="ExternalOutput")

    XCH = 2048  # xa streaming chunk (cols)

    with TileContext(nc) as tc:
        with (
            tc.tile_pool(name="const", bufs=1) as constp,
            tc.tile_pool(name="xring", bufs=4) as xringp,
            tc.tile_pool(name="stage", bufs=1) as stagep,
            tc.tile_pool(name="scratch", bufs=4) as scrp,
            tc.tile_pool(name="psum", bufs=8, space=bass.MemorySpace.PSUM) as psump,
        ):
            wb_sb = constp.tile([SIN + 1, SOUT], f32, tag="wb")
            nc.sync.dma_start(wb_sb[:], wb[:])
            u_sb = constp.tile([SOUT, 2], f32, tag="uu")
            nc.sync.dma_start(u_sb[:], uu[:])
            u0 = u_sb[:, 0:1]
            u1 = u_sb[:, 1:2]

            stage = stagep.tile([SOUT, NCOLS_G], f32, tag="stage")
            # zero-fill staging (gaps must read as 0); split so early
            # segments are ready fast. GpSimd keeps it off DVE/ACT.
            for s in range(NSEG):
                lo = s * SEG
                hi = min(lo + SEG, NCOLS_G)
                nc.gpsimd.memset(stage[:, lo:hi], 0.0)

            # xa streaming ring
            xtiles = [None] * (NCOLS // XCH)

            def load_x(k):
                if xtiles[k] is None:
                    t = xringp.tile([SIN + 1, XCH], f32, tag="xa")
                    nc.sync.dma_start(t[:], xa[:, k * XCH : (k + 1) * XCH])
                    xtiles[k] = t

            # GEMM chunk -> PSUM (diag-aligned, may span two xa tiles)
            pstile = [None] * len(chunks)

            def emit_chunk(ci):
                if pstile[ci] is not None:
                    return
                _, _, col0, ncols = chunks[ci]
                ps = psump.tile([SOUT, 512], f32, tag="ps")
                pos = 0
                while pos < ncols:
                    k = (col0 + pos) // XCH
                    load_x(k)
                    if k + 1 < len(xtiles):
                        load_x(k + 1)
                    take = min(ncols - pos, (k + 1) * XCH - (col0 + pos))
                    off = col0 + pos - k * XCH
                    nc.tensor.matmul(
                        out=ps[:, pos : pos + take],
                        lhsT=wb_sb[:],
                        rhs=xtiles[k][:, off : off + take],
                        start=True,
                        stop=True,
                    )
                    pos += take
                pstile[ci] = ps

            emit_chunk(0)
            emit_chunk(1)

            seg_done = 0
            for d in range(ND):
                i0, C = geo[d]
                n = C * BLOC
                gb = gbases[d]
                pgb = gbases[d - 1] if d > 0 else 0  # d=0: zeros at [0,GAP)
                ci = chunk_of_diag[d]
                emit_chunk(ci)
                if ci + 1 < len(chunks):
                    emit_chunk(ci + 1)
                ps = pstile[ci]
                poff = bases[d] * BLOC - chunks[ci][2]

                if d > 0 and geo[d - 1][0] < i0:
                    # shrinking phase: prev diag starts one cell lower
                    hls = pgb + GAP
                    hus = pgb
                else:
                    # growing: top boundary = gap before prev, bottom =
                    # gap after prev (both zero)
                    hls = pgb
                    hus = pgb - GAP if d > 0 else 0

                # split at absolute cell 64: half-a of diag d depends
                # only on half-a of d-1, so halves pipeline on engines.
                i1 = i0 + C - 1
                parts = []
                if i0 <= 63:
                    parts.append(("a", i0, min(i1, 63)))
                if i1 >= 64:
                    parts.append(("b", max(i0, 64), i1))
                for tagc, pa, pb in parts:
                    sz = (pb - pa + 1) * BLOC
                    off = (pa - i0) * BLOC
                    t1 = scrp.tile([SOUT, 256], f32, tag=f"t1{tagc}")
                    nc.vector.scalar_tensor_tensor(
                        out=t1[:, :sz],
                        in0=stage[:, hls + off : hls + off + sz],
                        scalar=u1,
                        in1=ps[:, poff + off : poff + off + sz],
                        op0=mult,
                        op1=add,
                    )
                    t2 = scrp.tile([SOUT, 256], f32, tag=f"t2{tagc}")
                    nc.vector.scalar_tensor_tensor(
                        out=t2[:, :sz],
                        in0=stage[:, hus + off : hus + off + sz],
                        scalar=u0,
                        in1=t1[:, :sz],
                        op0=mult,
                        op1=add,
                    )
                    nc.scalar.activation(
                        out=stage[:, gb + off : gb + off + sz],
                        in_=t2[:, :sz],
                        func=Tanh,
                    )
                # flush finished staging segments
                while (seg_done + 1) * SEG <= gb:
                    lo = seg_done * SEG
                    nc.sync.dma_start(ho[:, lo : lo + SEG], stage[:, lo : lo + SEG])
                    seg_done += 1
            while seg_done * SEG < NCOLS_G:
                lo = seg_done * SEG
                hi = min(lo + SEG, NCOLS_G)
                nc.sync.dma_start(ho[:, lo:hi], stage[:, lo:hi])
                seg_done += 1

    nc.compile()
    _CACHE["nc"] = nc
    return nc


def _prep_inputs(x, w, u, bias):
    I, J = _diag_order()
    xa_cells = np.ascontiguousarray(x[I, J])  # (16384, B, SIN)
    wbm = np.concatenate([w, bias[None, :]], axis=0).astype(np.float32)
    um = np.ascontiguousarray(u.T).astype(np.float32)  # (128,2): u0,u1 cols
    in_maps = []
    for c in range(NCORES):
        xc = xa_cells[:, c * BLOC : (c + 1) * BLOC, :]  # (16384, 2, 64)
        xc = xc.transpose(2, 0, 1).reshape(SIN, NCOLS)  # cell-major
        xc = np.concatenate([xc, np.ones((1, NCOLS), np.float32)], axis=0)
        in_maps.append({"xa": np.ascontiguousarray(xc), "wb": wbm, "uu": um})
    return in_maps


def _assemble(results):
    I, J = _diag_order()
    geo, bases, gbases = _diag_geom()
    valid = np.zeros(NCOLS, np.int64)
    for d in range(ND):
        n = geo[d][1] * BLOC
        valid[bases[d] * BLOC : bases[d] * BLOC + n] = gbases[d] + np.arange(n)
    out = np.zeros((D1, D2, B, SOUT), np.float32)
    for c in range(NCORES):
        hoc = results[c]["ho"][:, valid]  # (128, 32768) packed
        h_core = hoc.reshape(SOUT, NCELLS, BLOC).transpose(1, 2, 0)
        out[I, J, c * BLOC : (c + 1) * BLOC, :] = h_core
    return out


def kernel(x, w, u, bias, _trace=False):
    from concourse.bass_utils import run_bass_kernel_spmd

    x = np.asarray(x, dtype=np.float32)
    w = np.asarray(w, dtype=np.float32)
    u = np.asarray(u, dtype=np.float32)
    bias = np.asarray(bias, dtype=np.float32)

    nc = _build_program()
    in_maps = _prep_inputs(x, w, u, bias)
    res = run_bass_kernel_spmd(
        nc, in_maps, core_ids=list(range(NCORES)), trace=_trace
    )
    _CACHE["last_result"] = res
    return _assemble(res.results)

